# revision 10
# baseline (speedup 1.0000x reference)
"""Trainium2 Bass kernel for the quantum-control calibration loss.

Reference computation (per sample b of 2M):
    unitary[b] = prod_s exp(-i * DT*omega[b,s] * H)   (10 segments, same H)
    infid[b]   = 1 - |tr(sigma_x^H unitary[b])|^2 / 4
    loss       = mean((infedility_data[b] - infid[b])^2)

Because every step exponentiates the SAME Hamiltonian H, the factors commute
and the product collapses exactly:
    unitary[b] = exp(-i * Phi_b * H),   Phi_b = DT * sum_s omega[b,s]
With H = H0 traceless (by construction) and target = sigma_x (traceless):
    infid[b] = 1 - k*sin^2(r*Phi_b),  k = |tr(sigma_x H0)|^2 / (4 r^2)
    w_b      = d'_b + s_b,  d' = (2/k)*d + (1 - 2/k),
               s_b = sin(2*DT*r*sum_s omega[b,s] - pi/2) = -cos(2*r*Phi_b)
    loss     = (k^2/4) * mean(w_b^2)

Device strategy (pure data parallel over 8 cores, 250k rows each):
  - one blob per core holds, per tile, a [128, 12*ft] byte block per
    partition: 10 fp8 omega segment rows then one bf16 d' row (bitcast
    view). ~3.0 MB/core.
  - everything streams on the single sync HWDGE queue (a second queue makes
    the SDMA engines round-robin at packet granularity, costing ~180ns per
    switch; a single queue measured back-to-back at ~345 GB/s aggregate).
    Per-tile DMAs let compute track the stream; first tile small for an
    early compute start, last tile small for a short post-stream tail.
  - TensorE: 5 fp8 DoubleRow identity-matmul accumulates per tile (f32 PSUM,
    exact); redundant LDWEIGHTS are pruned from the BIR (identity stationary
    never changes; one load per tile keeps per-tile DMA waits intact).
    ~20 small dummy matmuls at program start walk the PE out of its low
    p-state (0.65/1.2 GHz ramp, 2.4 GHz only after ~3us continuously busy)
    so real matmuls run at full speed; sized to drain right when tile 0's
    data lands.
  - ScalarE runs only Sin (PSUM f32 -> bf16). VectorE does w = s + d'
    (bf16+bf16, 2x lanes), e2 = w*w (2x), and the per-tile reduce into f32
    partials (1x).
  - host sums the 8 x 128 x 5 partials in f64, applies k^2/4, divides by 2M.
"""

import math
from contextlib import ExitStack

import numpy as np

import concourse.bacc as bacc
import concourse.bass as bass
import concourse.tile as tile
from concourse import mybir
from concourse.bass_utils import run_bass_kernel_spmd

N_CORES = 8
NSEG = 10
ROWB = NSEG + 2          # bytes per sample per partition: 10 fp8 + bf16 d'
DT = 0.1
P = 128
# tile widths: multiples of 16 (DoubleRow AP step constraint); first tile
# smallish for an early compute start, last tile small for a short tail.
F_LIST = [240, 512, 512, 512, 192]
T = len(F_LIST)
F_TOT = sum(F_LIST)            # 1968 rows per partition
F_OFF = [sum(F_LIST[:i]) for i in range(T)]
R_PAD = P * F_TOT              # 251,904 rows per core
B_TOTAL = 2_000_000
B_LOCAL = B_TOTAL // N_CORES   # 250,000
N_WARMUP_MM = 30               # PE p-state warm-up matmuls (128 cols each)
N_FILLER_MM = 10               # dummy matmuls between tiles keep the PE's
                               # DVFS ramp alive while waiting on the stream

FP8 = mybir.dt.float8e4
BF16 = mybir.dt.bfloat16
NP_FP8 = mybir.dt.np(FP8)
NP_BF16 = mybir.dt.np(BF16)

HAM = np.array([[0.0, 0.5], [0.5, 0.0]], dtype=np.complex64)
TARGET = np.array([[0.0, 1.0], [1.0, 0.0]], dtype=np.complex64)

_STATE: dict = {}
LAST_RESULTS = None  # BassKernelResults of the most recent device run
NEG_HALFPI = float(np.float32(-np.pi / 2))


def _dedup_ldweights(nc, keep_names):
    """Drop InstLdweights whose name is not in keep_names.

    tile_legalize emits one LDWEIGHTS per fp8 matmul even when the stationary
    operand is identical; the PE array retains weights between matmuls, so
    one load per tile suffices (the per-tile load keeps that tile's
    DMA-completion wait on the PE stream)."""
    for b in nc.m.functions[0].blocks:
        insts = b.instructions
        rm = [
            i
            for i in insts
            if isinstance(i, mybir.InstLdweights) and i.name not in keep_names
        ]
        if not rm:
            continue
        for i in rm:
            for dep_name, _info in list(i.dependency_edges()):
                i.remove_dependency(dep_name)
        names = {i.name for i in rm}
        for idx in range(len(insts) - 1, -1, -1):
            if insts[idx].name in names:
                del insts[idx]


def _build_nc(two_c0: float) -> bass.Bass:
    nc = bacc.Bacc(None, target_bir_lowering=False, debug=False)
    f32 = mybir.dt.float32
    idp = nc.declare_dram_parameter("ident", [P, 2, P], FP8, isOutput=False)
    blob = nc.declare_dram_parameter("blob", [R_PAD * ROWB], FP8, isOutput=False)
    out = nc.declare_dram_parameter("partials", [P, T], f32, isOutput=True)

    first_mm_names = []
    with tile.TileContext(nc) as tc, ExitStack() as ctx:
        pool = ctx.enter_context(tc.tile_pool(name="pool", bufs=1))
        psump = ctx.enter_context(tc.tile_pool(name="psum", bufs=1, space="PSUM"))

        # per-tile blob DMAs first (tile 0 leads), then the tiny ident
        om_tiles = []
        dp_tiles = []
        dma_tiles = []
        base = 0
        for t in range(T):
            ft = F_LIST[t]
            w = ROWB * ft
            om_t = pool.tile([P, w], FP8, tag=f"om{t}")
            dma_tiles.append((om_t, base, w))
            base += P * w
            om_tiles.append(
                om_t[:, : NSEG * ft].rearrange("p (s f) -> p s f", s=NSEG, f=ft)
            )
            dp_tiles.append(om_t[:, NSEG * ft :].bitcast(BF16))
        ident_t = pool.tile([P, 2, P], FP8, tag="ident")

        om_t0, b0, w0 = dma_tiles[0]
        nc.sync.dma_start(
            out=om_t0, in_=blob[b0 : b0 + P * w0].rearrange("(p x) -> p x", p=P, x=w0)
        )
        nc.sync.dma_start(out=ident_t, in_=idp[:, :, :])
        for om_t, b, w in dma_tiles[1:]:
            nc.sync.dma_start(
                out=om_t, in_=blob[b : b + P * w].rearrange("(p x) -> p x", p=P, x=w)
            )

        # PE p-state warm-up: back-to-back dummy matmuls on zeroed scratch.
        scratch_w = pool.tile([P, 2, P], FP8, tag="wu_w")
        scratch_x = pool.tile([P, 2, 128], FP8, tag="wu_x")
        nc.vector.memset(scratch_w, 0.0)
        nc.vector.memset(scratch_x, 0.0)
        biasneg = pool.tile([P, 1], f32, tag="bias")
        nc.vector.memset(biasneg, NEG_HALFPI)
        acc = pool.tile([P, T], f32, tag="acc")

        wu_psum = psump.tile([P, 128], f32, tag="wu_ps")
        for j in range(N_WARMUP_MM):
            h = nc.tensor.matmul(
                wu_psum,
                scratch_w,
                scratch_x,
                start=True,
                stop=True,
                perf_mode=mybir.MatmulPerfMode.DoubleRow,
                skip_group_check=True,
            )
            if j == 0:
                first_mm_names.append(h.ins.name)

        for t in range(T):
            ft = F_LIST[t]
            om_t = om_tiles[t]
            rs = psump.tile([P, ft], f32, tag=f"rs{t}")
            for j in range(NSEG // 2):
                h = nc.tensor.matmul(
                    rs,
                    ident_t,
                    om_t[:, 2 * j : 2 * j + 2, :],
                    start=(j == 0),
                    stop=(j == NSEG // 2 - 1),
                    perf_mode=mybir.MatmulPerfMode.DoubleRow,
                )
                if j == 0:
                    first_mm_names.append(h.ins.name)
            if t < T - 1:
                # keep the PE busy (and its DVFS ramp alive) until the next
                # tile's data lands; these drop straight through when the
                # stream is ahead.
                for j in range(N_FILLER_MM):
                    h = nc.tensor.matmul(
                        wu_psum,
                        scratch_w,
                        scratch_x,
                        start=True,
                        stop=True,
                        perf_mode=mybir.MatmulPerfMode.DoubleRow,
                        skip_group_check=True,
                    )
                    if j == 0:
                        first_mm_names.append(h.ins.name)
            # s = sin(two_c0*rs - pi/2) = -cos(2*theta), bf16
            s_t = pool.tile([P, ft], BF16, tag=f"s{t}")
            nc.scalar.activation(
                out=s_t,
                in_=rs,
                func=mybir.ActivationFunctionType.Sin,
                scale=two_c0,
                bias=biasneg,
            )
            # w = s + d' (bf16, 2x); e2 = w*w (bf16, 2x); acc[:, t] = sum e2
            w_t = pool.tile([P, ft], BF16, tag=f"w{t}")
            nc.vector.tensor_tensor(
                out=w_t, in0=s_t, in1=dp_tiles[t], op=mybir.AluOpType.add
            )
            e2 = pool.tile([P, ft], BF16, tag=f"e2{t}")
            nc.vector.tensor_tensor(
                out=e2, in0=w_t, in1=w_t, op=mybir.AluOpType.mult
            )
            nc.vector.tensor_reduce(
                out=acc[:, t : t + 1],
                in_=e2,
                axis=mybir.AxisListType.X,
                op=mybir.AluOpType.add,
            )
            # per-tile result flush: all but the last column's DMA receipt
            # latency hides behind later work
            nc.sync.dma_start(out=out[:, t : t + 1], in_=acc[:, t : t + 1])

    # prune redundant LDWEIGHTS: keep the warm-up's load and the one
    # preceding each tile's first matmul
    firsts = set(first_mm_names)
    keep = set()
    for b in nc.m.functions[0].blocks:
        prev_ldw = None
        for i in b.instructions:
            if isinstance(i, mybir.InstLdweights):
                prev_ldw = i.name
            elif isinstance(i, mybir.InstMatmult) and i.name in firsts:
                if prev_ldw is not None:
                    keep.add(prev_ldw)
    _dedup_ldweights(nc, keep)

    nc.compile()
    return nc


def _scalar_params(x: np.ndarray):
    """Mimic the reference's f32/complex64 scalar preprocessing of the 2x2."""
    eye = np.eye(2, dtype=np.complex64)
    xc = np.asarray(x, dtype=np.float32).astype(np.complex64)
    herm = (xc + xc.T) * np.complex64(0.5) + np.complex64(1j) * (xc - xc.T) * np.complex64(0.5)
    ham_unknown = herm - np.trace(herm) * eye / np.complex64(2)
    H = HAM + ham_unknown
    tr = np.trace(H)
    H0 = H - tr * eye / np.complex64(2)
    rsq = float(np.einsum("ij,ji->", H0, H0).real) / 2.0
    r = math.sqrt(max(rsq, 1e-30))
    M = complex((TARGET.conj() * H0).sum())
    k = (abs(M) ** 2) / (4.0 * rsq) if rsq > 0 else 0.0
    return rsq, r, k


def _numpy_reference(x, omega, d):
    """Literal f32 fallback for the degenerate rsq<=1e-24 branch (never taken
    for realistic inputs; kept for exact semantic coverage)."""
    eye = np.eye(2, dtype=np.complex64)
    xc = np.asarray(x, dtype=np.float32).astype(np.complex64)
    herm = (xc + xc.T) * np.complex64(0.5) + np.complex64(1j) * (xc - xc.T) * np.complex64(0.5)
    ham_unknown = herm - np.trace(herm) * eye / np.complex64(2)
    H = HAM + ham_unknown
    tr = np.trace(H)
    H0 = H - tr * eye / np.complex64(2)
    rsq = np.float32(np.einsum("ij,ji->", H0, H0).real / 2)
    r = np.sqrt(np.maximum(rsq, np.float32(1e-30)))
    B = omega.shape[0]
    u = np.broadcast_to(eye, (B, 2, 2)).copy()
    for s in range(NSEG):
        phi = (np.float32(DT) * omega[:, s]).astype(np.float32)
        theta = phi * r
        sinc = np.where(rsq > 1e-24, np.sin(theta) / r, phi)
        phase = np.exp(np.complex64(-1j) * phi.astype(np.complex64) * tr / 2)
        u_step = phase[:, None, None] * (
            np.cos(theta).astype(np.complex64)[:, None, None] * eye
            - np.complex64(1j) * sinc.astype(np.complex64)[:, None, None] * H0
        )
        u = np.einsum("bij,bjk->bik", u_step, u)
    tmp0 = (TARGET.conj()[None] * u).sum(axis=(1, 2))
    infid = 1.0 - (tmp0 * tmp0.conj()).real / 4
    return np.float32(np.mean((d - infid) ** 2))


def kernel(para_ham_unknown, omega_data, infedility_data):
    global LAST_RESULTS
    x = np.asarray(para_ham_unknown, dtype=np.float32)
    omega = np.ascontiguousarray(np.asarray(omega_data, dtype=np.float32))
    d = np.ascontiguousarray(np.asarray(infedility_data, dtype=np.float32))

    rsq, r, k = _scalar_params(x)
    if rsq <= 1e-24:
        return _numpy_reference(x, omega, d)

    two_c0 = float(np.float32(2.0 * DT * r))

    B = omega.shape[0]
    assert B == B_TOTAL, f"kernel compiled for B={B_TOTAL}, got {B}"

    # shard + pad: padded rows have omega=0, d'=1 -> s=-1, w=0, e=0
    # row within a core = P*F_OFF[t] + p*F_LIST[t] + f; device block per tile
    # is [P, 12*ft] bytes (10 fp8 omega rows + 1 bf16 d' row).
    om_pad = np.zeros((N_CORES, R_PAD, NSEG), dtype=NP_FP8)
    om_pad[:, :B_LOCAL, :] = omega.reshape(N_CORES, B_LOCAL, NSEG).astype(NP_FP8)
    dp = np.ones((N_CORES, R_PAD), dtype=NP_BF16)
    dp[:, :B_LOCAL] = (
        (2.0 / k) * d.reshape(N_CORES, B_LOCAL) + (1.0 - 2.0 / k)
    ).astype(NP_BF16)

    blob = np.empty((N_CORES, R_PAD * ROWB), dtype=np.uint8)
    off = 0
    for t in range(T):
        ft = F_LIST[t]
        rows = slice(P * F_OFF[t], P * (F_OFF[t] + ft))
        blk = np.empty((N_CORES, P, ROWB * ft), dtype=np.uint8)
        blk[:, :, : NSEG * ft] = (
            om_pad[:, rows, :]
            .view(np.uint8)
            .reshape(N_CORES, P, ft, NSEG)
            .transpose(0, 1, 3, 2)
            .reshape(N_CORES, P, NSEG * ft)
        )
        blk[:, :, NSEG * ft :] = (
            dp[:, rows].reshape(N_CORES, P, ft).view(np.uint8)
        )
        blob[:, off : off + P * ROWB * ft] = blk.reshape(N_CORES, -1)
        off += P * ROWB * ft
    blob = blob.view(NP_FP8)

    ident = np.broadcast_to(np.eye(P, dtype=NP_FP8)[:, None, :], (P, 2, P)).copy()

    key = (two_c0,)
    if _STATE.get("key") != key:
        _STATE["nc"] = _build_nc(*key)
        _STATE["key"] = key
    nc = _STATE["nc"]

    in_maps = [{"blob": blob[i], "ident": ident} for i in range(N_CORES)]
    res = run_bass_kernel_spmd(nc, in_maps, core_ids=list(range(N_CORES)))
    LAST_RESULTS = res

    total = 0.0
    for core_res in res.results:
        total += float(core_res["partials"].astype(np.float64).sum())
    return np.float32((k * k / 4.0) * total / B_TOTAL)


# revision 11
# speedup vs baseline: 1.0258x; 1.0258x over previous
"""Trainium2 Bass kernel for the quantum-control calibration loss.

Reference computation (per sample b of 2M):
    unitary[b] = prod_s exp(-i * DT*omega[b,s] * H)   (10 segments, same H)
    infid[b]   = 1 - |tr(sigma_x^H unitary[b])|^2 / 4
    loss       = mean((infedility_data[b] - infid[b])^2)

Because every step exponentiates the SAME Hamiltonian H, the factors commute
and the product collapses exactly:
    unitary[b] = exp(-i * Phi_b * H),   Phi_b = DT * sum_s omega[b,s]
With H = H0 traceless (by construction) and target = sigma_x (traceless):
    infid[b] = 1 - k*sin^2(r*Phi_b),  k = |tr(sigma_x H0)|^2 / (4 r^2)
    w_b      = d'_b + s_b,  d' = (2/k)*d + (1 - 2/k),
               s_b = sin(2*DT*r*sum_s omega[b,s] - pi/2) = -cos(2*r*Phi_b)
    loss     = (k^2/4) * mean(w_b^2)

Device strategy (pure data parallel over 8 cores, 250k rows each):
  - one blob per core holds, per tile, a [128, 12*ft] byte block per
    partition: 10 fp8 omega segment rows then one bf16 d' row (bitcast
    view). ~3.0 MB/core.
  - everything streams on the single sync HWDGE queue (a second queue makes
    the SDMA engines round-robin at packet granularity, costing ~180ns per
    switch; a single queue measured back-to-back at ~345 GB/s aggregate).
    Per-tile DMAs let compute track the stream; first tile small for an
    early compute start, last tile small for a short post-stream tail.
  - TensorE: 5 fp8 DoubleRow identity-matmul accumulates per tile (f32 PSUM,
    exact); redundant LDWEIGHTS are pruned from the BIR (identity stationary
    never changes; one load per tile keeps per-tile DMA waits intact).
    ~20 small dummy matmuls at program start walk the PE out of its low
    p-state (0.65/1.2 GHz ramp, 2.4 GHz only after ~3us continuously busy)
    so real matmuls run at full speed; sized to drain right when tile 0's
    data lands.
  - ScalarE runs only Sin (PSUM f32 -> bf16). VectorE does w = s + d'
    (bf16+bf16, 2x lanes), e2 = w*w (2x), and the per-tile reduce into f32
    partials (1x).
  - host sums the 8 x 128 x 5 partials in f64, applies k^2/4, divides by 2M.
"""

import math
from contextlib import ExitStack

import numpy as np

import concourse.bacc as bacc
import concourse.bass as bass
import concourse.tile as tile
from concourse import mybir
from concourse.bass_utils import run_bass_kernel_spmd

N_CORES = 8
NSEG = 10
ROWB = NSEG + 2          # bytes per sample per partition: 10 fp8 + bf16 d'
DT = 0.1
P = 128
# tile widths: multiples of 16 (DoubleRow AP step constraint); first tile
# smallish for an early compute start, last tile small for a short tail.
F_LIST = [240, 512, 512, 512, 192]
T = len(F_LIST)
F_TOT = sum(F_LIST)            # 1968 rows per partition
F_OFF = [sum(F_LIST[:i]) for i in range(T)]
R_PAD = P * F_TOT              # 251,904 rows per core
B_TOTAL = 2_000_000
B_LOCAL = B_TOTAL // N_CORES   # 250,000
N_WARMUP_MM = 30               # PE p-state warm-up matmuls (128 cols each)
N_FILLER_MM = 10               # dummy matmuls between tiles keep the PE's
                               # DVFS ramp alive while waiting on the stream

FP8 = mybir.dt.float8e4
BF16 = mybir.dt.bfloat16
NP_FP8 = mybir.dt.np(FP8)
NP_BF16 = mybir.dt.np(BF16)

HAM = np.array([[0.0, 0.5], [0.5, 0.0]], dtype=np.complex64)
TARGET = np.array([[0.0, 1.0], [1.0, 0.0]], dtype=np.complex64)

_STATE: dict = {}
LAST_RESULTS = None  # BassKernelResults of the most recent device run
NEG_HALFPI = float(np.float32(-np.pi / 2))


def _dedup_ldweights(nc, keep_names):
    """Drop InstLdweights whose name is not in keep_names.

    tile_legalize emits one LDWEIGHTS per fp8 matmul even when the stationary
    operand is identical; the PE array retains weights between matmuls, so
    one load per tile suffices (the per-tile load keeps that tile's
    DMA-completion wait on the PE stream)."""
    for b in nc.m.functions[0].blocks:
        insts = b.instructions
        rm = [
            i
            for i in insts
            if isinstance(i, mybir.InstLdweights) and i.name not in keep_names
        ]
        if not rm:
            continue
        for i in rm:
            for dep_name, _info in list(i.dependency_edges()):
                i.remove_dependency(dep_name)
        names = {i.name for i in rm}
        for idx in range(len(insts) - 1, -1, -1):
            if insts[idx].name in names:
                del insts[idx]


def _build_nc(two_c0: float) -> bass.Bass:
    nc = bacc.Bacc(None, target_bir_lowering=False, debug=False)
    f32 = mybir.dt.float32
    idp = nc.declare_dram_parameter("ident", [P, 2, P], FP8, isOutput=False)
    blob = nc.declare_dram_parameter("blob", [R_PAD * ROWB], FP8, isOutput=False)
    out = nc.declare_dram_parameter("partials", [P, T], f32, isOutput=True)

    first_mm_names = []
    with tile.TileContext(nc) as tc, ExitStack() as ctx:
        pool = ctx.enter_context(tc.tile_pool(name="pool", bufs=1))
        psump = ctx.enter_context(tc.tile_pool(name="psum", bufs=1, space="PSUM"))

        # per-tile blob DMAs first (tile 0 leads), then the tiny ident
        om_tiles = []
        dp_tiles = []
        dma_tiles = []
        base = 0
        for t in range(T):
            ft = F_LIST[t]
            w = ROWB * ft
            om_t = pool.tile([P, w], FP8, tag=f"om{t}")
            dma_tiles.append((om_t, base, w))
            base += P * w
            om_tiles.append(
                om_t[:, : NSEG * ft].rearrange("p (s f) -> p s f", s=NSEG, f=ft)
            )
            dp_tiles.append(om_t[:, NSEG * ft :].bitcast(BF16))
        ident_t = pool.tile([P, 2, P], FP8, tag="ident")

        om_t0, b0, w0 = dma_tiles[0]
        nc.sync.dma_start(
            out=om_t0, in_=blob[b0 : b0 + P * w0].rearrange("(p x) -> p x", p=P, x=w0)
        )
        nc.sync.dma_start(out=ident_t, in_=idp[:, :, :])
        for om_t, b, w in dma_tiles[1:]:
            nc.sync.dma_start(
                out=om_t, in_=blob[b : b + P * w].rearrange("(p x) -> p x", p=P, x=w)
            )

        # PE p-state warm-up: back-to-back dummy matmuls on zeroed scratch.
        scratch_w = pool.tile([P, 2, P], FP8, tag="wu_w")
        scratch_x = pool.tile([P, 2, 128], FP8, tag="wu_x")
        nc.vector.memset(scratch_w, 0.0)
        nc.vector.memset(scratch_x, 0.0)
        biasneg = pool.tile([P, 1], f32, tag="bias")
        nc.vector.memset(biasneg, NEG_HALFPI)
        acc = pool.tile([P, T], f32, tag="acc")

        wu_psum = psump.tile([P, 128], f32, tag="wu_ps")
        for j in range(N_WARMUP_MM):
            h = nc.tensor.matmul(
                wu_psum,
                scratch_w,
                scratch_x,
                start=True,
                stop=True,
                perf_mode=mybir.MatmulPerfMode.DoubleRow,
                skip_group_check=True,
            )
            if j == 0:
                first_mm_names.append(h.ins.name)

        for t in range(T):
            ft = F_LIST[t]
            om_t = om_tiles[t]
            rs = psump.tile([P, ft], f32, tag=f"rs{t}")
            for j in range(NSEG // 2):
                h = nc.tensor.matmul(
                    rs,
                    ident_t,
                    om_t[:, 2 * j : 2 * j + 2, :],
                    start=(j == 0),
                    stop=(j == NSEG // 2 - 1),
                    perf_mode=mybir.MatmulPerfMode.DoubleRow,
                )
                if j == 0:
                    first_mm_names.append(h.ins.name)
            if t < T - 1:
                # keep the PE busy (and its DVFS ramp alive) until the next
                # tile's data lands; these drop straight through when the
                # stream is ahead.
                for j in range(N_FILLER_MM):
                    h = nc.tensor.matmul(
                        wu_psum,
                        scratch_w,
                        scratch_x,
                        start=True,
                        stop=True,
                        perf_mode=mybir.MatmulPerfMode.DoubleRow,
                        skip_group_check=True,
                    )
                    if j == 0:
                        first_mm_names.append(h.ins.name)
            # s = sin(two_c0*rs - pi/2) = -cos(2*theta), bf16
            s_t = pool.tile([P, ft], BF16, tag=f"s{t}")
            nc.scalar.activation(
                out=s_t,
                in_=rs,
                func=mybir.ActivationFunctionType.Sin,
                scale=two_c0,
                bias=biasneg,
            )
            # w = s + d' (bf16, 2x); e2 = w*w (bf16, 2x); acc[:, t] = sum e2
            w_t = pool.tile([P, ft], BF16, tag=f"w{t}")
            nc.vector.tensor_tensor(
                out=w_t, in0=s_t, in1=dp_tiles[t], op=mybir.AluOpType.add
            )
            e2 = pool.tile([P, ft], BF16, tag=f"e2{t}")
            nc.vector.tensor_tensor(
                out=e2, in0=w_t, in1=w_t, op=mybir.AluOpType.mult
            )
            nc.vector.tensor_reduce(
                out=acc[:, t : t + 1],
                in_=e2,
                axis=mybir.AxisListType.X,
                op=mybir.AluOpType.add,
            )

        nc.sync.dma_start(out=out[:, :], in_=acc)

    # prune redundant LDWEIGHTS: keep the warm-up's load and the one
    # preceding each tile's first matmul
    firsts = set(first_mm_names)
    keep = set()
    for b in nc.m.functions[0].blocks:
        prev_ldw = None
        for i in b.instructions:
            if isinstance(i, mybir.InstLdweights):
                prev_ldw = i.name
            elif isinstance(i, mybir.InstMatmult) and i.name in firsts:
                if prev_ldw is not None:
                    keep.add(prev_ldw)
    _dedup_ldweights(nc, keep)

    nc.compile()
    return nc


def _scalar_params(x: np.ndarray):
    """Mimic the reference's f32/complex64 scalar preprocessing of the 2x2."""
    eye = np.eye(2, dtype=np.complex64)
    xc = np.asarray(x, dtype=np.float32).astype(np.complex64)
    herm = (xc + xc.T) * np.complex64(0.5) + np.complex64(1j) * (xc - xc.T) * np.complex64(0.5)
    ham_unknown = herm - np.trace(herm) * eye / np.complex64(2)
    H = HAM + ham_unknown
    tr = np.trace(H)
    H0 = H - tr * eye / np.complex64(2)
    rsq = float(np.einsum("ij,ji->", H0, H0).real) / 2.0
    r = math.sqrt(max(rsq, 1e-30))
    M = complex((TARGET.conj() * H0).sum())
    k = (abs(M) ** 2) / (4.0 * rsq) if rsq > 0 else 0.0
    return rsq, r, k


def _numpy_reference(x, omega, d):
    """Literal f32 fallback for the degenerate rsq<=1e-24 branch (never taken
    for realistic inputs; kept for exact semantic coverage)."""
    eye = np.eye(2, dtype=np.complex64)
    xc = np.asarray(x, dtype=np.float32).astype(np.complex64)
    herm = (xc + xc.T) * np.complex64(0.5) + np.complex64(1j) * (xc - xc.T) * np.complex64(0.5)
    ham_unknown = herm - np.trace(herm) * eye / np.complex64(2)
    H = HAM + ham_unknown
    tr = np.trace(H)
    H0 = H - tr * eye / np.complex64(2)
    rsq = np.float32(np.einsum("ij,ji->", H0, H0).real / 2)
    r = np.sqrt(np.maximum(rsq, np.float32(1e-30)))
    B = omega.shape[0]
    u = np.broadcast_to(eye, (B, 2, 2)).copy()
    for s in range(NSEG):
        phi = (np.float32(DT) * omega[:, s]).astype(np.float32)
        theta = phi * r
        sinc = np.where(rsq > 1e-24, np.sin(theta) / r, phi)
        phase = np.exp(np.complex64(-1j) * phi.astype(np.complex64) * tr / 2)
        u_step = phase[:, None, None] * (
            np.cos(theta).astype(np.complex64)[:, None, None] * eye
            - np.complex64(1j) * sinc.astype(np.complex64)[:, None, None] * H0
        )
        u = np.einsum("bij,bjk->bik", u_step, u)
    tmp0 = (TARGET.conj()[None] * u).sum(axis=(1, 2))
    infid = 1.0 - (tmp0 * tmp0.conj()).real / 4
    return np.float32(np.mean((d - infid) ** 2))


def kernel(para_ham_unknown, omega_data, infedility_data):
    global LAST_RESULTS
    x = np.asarray(para_ham_unknown, dtype=np.float32)
    omega = np.ascontiguousarray(np.asarray(omega_data, dtype=np.float32))
    d = np.ascontiguousarray(np.asarray(infedility_data, dtype=np.float32))

    rsq, r, k = _scalar_params(x)
    if rsq <= 1e-24:
        return _numpy_reference(x, omega, d)

    two_c0 = float(np.float32(2.0 * DT * r))

    B = omega.shape[0]
    assert B == B_TOTAL, f"kernel compiled for B={B_TOTAL}, got {B}"

    # shard + pad: padded rows have omega=0, d'=1 -> s=-1, w=0, e=0
    # row within a core = P*F_OFF[t] + p*F_LIST[t] + f; device block per tile
    # is [P, 12*ft] bytes (10 fp8 omega rows + 1 bf16 d' row).
    om_pad = np.zeros((N_CORES, R_PAD, NSEG), dtype=NP_FP8)
    om_pad[:, :B_LOCAL, :] = omega.reshape(N_CORES, B_LOCAL, NSEG).astype(NP_FP8)
    dp = np.ones((N_CORES, R_PAD), dtype=NP_BF16)
    dp[:, :B_LOCAL] = (
        (2.0 / k) * d.reshape(N_CORES, B_LOCAL) + (1.0 - 2.0 / k)
    ).astype(NP_BF16)

    blob = np.empty((N_CORES, R_PAD * ROWB), dtype=np.uint8)
    off = 0
    for t in range(T):
        ft = F_LIST[t]
        rows = slice(P * F_OFF[t], P * (F_OFF[t] + ft))
        blk = np.empty((N_CORES, P, ROWB * ft), dtype=np.uint8)
        blk[:, :, : NSEG * ft] = (
            om_pad[:, rows, :]
            .view(np.uint8)
            .reshape(N_CORES, P, ft, NSEG)
            .transpose(0, 1, 3, 2)
            .reshape(N_CORES, P, NSEG * ft)
        )
        blk[:, :, NSEG * ft :] = (
            dp[:, rows].reshape(N_CORES, P, ft).view(np.uint8)
        )
        blob[:, off : off + P * ROWB * ft] = blk.reshape(N_CORES, -1)
        off += P * ROWB * ft
    blob = blob.view(NP_FP8)

    ident = np.broadcast_to(np.eye(P, dtype=NP_FP8)[:, None, :], (P, 2, P)).copy()

    key = (two_c0,)
    if _STATE.get("key") != key:
        _STATE["nc"] = _build_nc(*key)
        _STATE["key"] = key
    nc = _STATE["nc"]

    in_maps = [{"blob": blob[i], "ident": ident} for i in range(N_CORES)]
    res = run_bass_kernel_spmd(nc, in_maps, core_ids=list(range(N_CORES)))
    LAST_RESULTS = res

    total = 0.0
    for core_res in res.results:
        total += float(core_res["partials"].astype(np.float64).sum())
    return np.float32((k * k / 4.0) * total / B_TOTAL)
